# revision 2
# baseline (speedup 1.0000x reference)
"""MoE (AriaExperts) Trainium2 kernel — expert parallelism across 8 NeuronCores.

Strategy:
  - Host: top-2 routing + softmax over [2048, 8] logits (tiny), build the
    per-expert token batches (the "all-to-all" is realized at input
    distribution time), and the weighted scatter-add combine at the end.
  - Device (SPMD, 1 expert per core): dense GEMM chain in transposed
    activation layout so both matmuls consume the expert weights directly
    as the stationary (lhsT) operand with zero on-device transposes:
        H^T  = W1^T-tiles @ X^T      [2*INTER, C]
        actT = silu(projT) * gateT   [INTER, C]
        outT = W2-tiles   @ actT     [HIDDEN, C]
    f32 PSUM accumulation; 1 cycle/row operand dtypes throughout.

  v2 changes (trace-driven, baseline 102.4 us with last-MM at 97.6 us and
  first real MM at 14.1 us):
    - w1 k-tiles 0-3 are shipped and consumed as e3m4 fp8 (4-bit mantissa,
      1 cycle/row like bf16, FWL still applies); k-tiles 4-7 stay bf16.
      Both halves are pre-scaled by 128 on the host (lossless for bf16,
      centers e3m4's tiny exponent range) so one PSUM chain mixes them;
      the 1/128 descale rides the silu input-scale and the FC2 output
      copy multiplier — zero extra ops. Halves the early w1 stream
      (8 -> 6 MB total input per core); numerically simulated end-to-end
      rel-err 1.40e-2 vs the 2e-2 gate (bf16 baseline 4.4e-3).
    - Fine-grained head: xt is delivered per k-tile (128 KB) and the
      first four w1 slots (pairs 0-1) per k-tile in a kt-major head
      layout (w1ah/w1bh), in exact consumption order. Pairs 0-1 run
      proj/gate interleaved per-kt so every arriving slice is consumed
      immediately -> first real matmul ~9 us instead of 14.1 (the first
      ~14 us are otherwise DMA-ramp-bound; the framework preamble alone
      is ~7 us before any doorbell can ring).
    - Warmup matmuls on a memset tile are interleaved into the fine
      phase to fill DMA-wait gaps: keeps the HAM activity window busy so
      the 2.4 GHz un-throttle lands before the PE-bound phase, without
      delaying data-ready real matmuls.
    - Engine split: scalar gets only 4 early fine doorbells + w2 half
      (frees it for the 16 silus from ~13 us); gpsimd carries the
      kt-major bf16 head (needed ~11 us) + the other w2 half; sync
      streams everything else in pair order. FC2 PSUM->SBUF copies live
      on DVE; output DMA triggers on scalar (lanes provably fresh).
    - PSUM pool = 8 x 1-bank [128,512] slots so 4 FC1 proj/gate pairs
      can be in flight; SwiGLU readout lags PE without blocking PSUM
      recycling.
    - Final FC2 m-tile splits into two independent half-column chains so
      the first half drains (copy+DMA) while the PE runs the second.
"""

import time

import numpy as np
import ml_dtypes

import concourse.bass as bass
import concourse.bacc as bacc
import concourse.mybir as mybir
import concourse.tile as tile
from concourse.bass_utils import run_bass_kernel_spmd

NUM_TOKENS = 2048
HIDDEN = 1024
INTER = 2048
NUM_EXPERTS = 8
TOPK = 2
NCORES = 8
P = 128
KT1 = HIDDEN // P         # 8  k-tiles (FC1 contraction)
KTA = KT1 // 2            # 4  e3m4 k-tiles (kt 0-3)
MT1 = 2 * INTER // P      # 32 m-tiles (FC1 output rows = proj+gate)
MT1H = INTER // P         # 16 proj/gate pair count
NHEAD = 4                 # head slots (pairs 0-1) in kt-major layout
KT2 = INTER // P          # 16 k-tiles (FC2 contraction)
MT2 = HIDDEN // P         # 8  m-tiles (FC2 output rows)
W1SCALE = 128.0           # host-side w1 pre-scale (both halves)

BF16 = mybir.dt.bfloat16
F8E3 = mybir.dt.float8e3
F32 = mybir.dt.float32
np_bf16 = ml_dtypes.bfloat16
np_e3m4 = ml_dtypes.float8_e3m4

# [0, 16, 1, 17, ...] — interleave proj/gate m-tiles into adjacent pairs
_W1_ORDER = np.arange(MT1).reshape(2, MT1H).T.reshape(-1)

NWARM_PRE = 3   # dummy matmuls before the first real one (~8.5 -> ~9.8 us
                # at the cold clock; first data lands ~9 us)
NWARM_MID = 6   # dummy matmuls interleaved after pair-0's kt 0..5 pairs to
                # fill DMA-pacing gaps and keep the HAM window busy

_graph_cache: dict = {}


def _build(NCH: int, CH: int) -> bass.Bass:
    """Per-core Bass graph for capacity C_pad = NCH * CH (CH <= 512)."""
    nc = bacc.Bacc("TRN2", target_bir_lowering=False, debug=False)

    xt_d = nc.declare_dram_parameter("xt", [P, KT1, NCH, CH], BF16, isOutput=False)
    # w1 split: kt 0-3 e3m4 / kt 4-7 bf16; head (slots 0-3) kt-major for
    # fine-grained arrival-order consumption, rest slot-major.
    w1ah_d = nc.declare_dram_parameter("w1ah", [P, KTA, NHEAD, P], F8E3, isOutput=False)
    w1ar_d = nc.declare_dram_parameter(
        "w1ar", [P, MT1 - NHEAD, KTA, P], F8E3, isOutput=False
    )
    w1bh_d = nc.declare_dram_parameter("w1bh", [P, KTA, NHEAD, P], BF16, isOutput=False)
    w1br_d = nc.declare_dram_parameter(
        "w1br", [P, MT1 - NHEAD, KTA, P], BF16, isOutput=False
    )
    w2_d = nc.declare_dram_parameter("w2", [P, MT2, KT2, P], BF16, isOutput=False)
    # bf16 output: halves the output DMA on the kernel tail; the host-side
    # combine upcasts to f32 (adds ~0.2% rounding — well within the gate).
    out_d = nc.declare_dram_parameter("out", [MT2, NCH, P, CH], BF16, isOutput=True)

    inv_scale = 1.0 / W1SCALE

    with tile.TileContext(nc) as tc:
        with (
            tc.tile_pool(name="weights", bufs=1) as wpool,
            tc.tile_pool(name="xin", bufs=1) as xpool,
            tc.tile_pool(name="actp", bufs=2) as apool,
            tc.tile_pool(name="tmp", bufs=4) as tpool,
            tc.tile_pool(name="osb", bufs=4) as opool,
            tc.tile_pool(name="psum", bufs=8, space="PSUM") as pspool,
        ):
            xt = xpool.tile([P, KT1, NCH, CH], BF16, tag="xt")
            w1ah = wpool.tile([P, KTA, NHEAD, P], F8E3, tag="w1ah")
            w1ar = wpool.tile([P, MT1 - NHEAD, KTA, P], F8E3, tag="w1ar")
            w1bh = wpool.tile([P, KTA, NHEAD, P], BF16, tag="w1bh")
            w1br = wpool.tile([P, MT1 - NHEAD, KTA, P], BF16, tag="w1br")
            w2 = wpool.tile([P, MT2, KT2, P], BF16, tag="w2")
            dummy = xpool.tile([P, 640], BF16, tag="dummy")

            # Memset on DVE (idle at boot; starts the warmups earliest).
            nc.vector.memset(dummy[:], 0.0)

            # Force the SILU activation-table load (~1.3 us, non-blocking)
            # right at engine boot, overlapped with the first input DMAs.
            tdum = tpool.tile([P, 8], F32, tag="tmp", name="tdum")
            nc.scalar.activation(
                tdum[:], dummy[:, :8], mybir.ActivationFunctionType.Silu
            )

            # ---- input DMA triggers ----
            # Fine head in exact consumption order of pairs 0-1 (proj/gate
            # interleaved per kt): xt k-tile then the matching head weight
            # slice. Evens on sync, odds on scalar (scalar stops after kt3
            # so its queue is clear for silus); bf16 head k-tiles (needed
            # ~11 us) and one w2 half ride gpsimd's SWDGE; sync carries the
            # rest in pair order.
            nc.sync.dma_start(out=xt[:, 0:1, 0], in_=xt_d[:, 0:1, 0])
            nc.scalar.dma_start(out=xt[:, 1:2, 0], in_=xt_d[:, 1:2, 0])
            nc.sync.dma_start(out=w1ah[:, 0:1], in_=w1ah_d[:, 0:1])
            nc.scalar.dma_start(out=w1ah[:, 1:2], in_=w1ah_d[:, 1:2])
            nc.sync.dma_start(out=xt[:, 2:3, 0], in_=xt_d[:, 2:3, 0])
            nc.scalar.dma_start(out=xt[:, 3:4, 0], in_=xt_d[:, 3:4, 0])
            nc.sync.dma_start(out=w1ah[:, 2:3], in_=w1ah_d[:, 2:3])
            nc.scalar.dma_start(out=w1ah[:, 3:4], in_=w1ah_d[:, 3:4])
            nc.gpsimd.dma_start(out=w1bh[:, 0:2], in_=w1bh_d[:, 0:2])
            nc.sync.dma_start(out=xt[:, 4:5, 0], in_=xt_d[:, 4:5, 0])
            nc.sync.dma_start(out=xt[:, 5:6, 0], in_=xt_d[:, 5:6, 0])
            nc.gpsimd.dma_start(out=w1bh[:, 2:4], in_=w1bh_d[:, 2:4])
            nc.sync.dma_start(out=xt[:, 6:7, 0], in_=xt_d[:, 6:7, 0])
            nc.sync.dma_start(out=xt[:, 7:8, 0], in_=xt_d[:, 7:8, 0])
            # Chunky rest in pair order: per 2-pair group, the e3m4 slots
            # then the bf16 slots. One w2 half early on scalar (its lanes
            # are long-fresh before the FC2 output triggers), the other on
            # gpsimd.
            nc.scalar.dma_start(out=w2[:, : MT2 // 2], in_=w2_d[:, : MT2 // 2])
            for g in range(0, MT1 - NHEAD, 4):
                nc.sync.dma_start(out=w1ar[:, g : g + 4], in_=w1ar_d[:, g : g + 4])
                nc.sync.dma_start(out=w1br[:, g : g + 4], in_=w1br_d[:, g : g + 4])
            nc.gpsimd.dma_start(out=w2[:, MT2 // 2 :], in_=w2_d[:, MT2 // 2 :])
            for ci in range(1, NCH):
                nc.sync.dma_start(out=xt[:, :, ci], in_=xt_d[:, :, ci])

            # PE warmup on the memset tile while the first slices stream in.
            warm_ps = pspool.tile([P, 512], F32, tag="ps", name="warmps")

            def warm(n):
                for _ in range(n):
                    nc.tensor.matmul(
                        warm_ps[:, :], dummy[:, :128], dummy[:, 128:640],
                        start=True, stop=True,
                    )

            warm(NWARM_PRE)

            def lhsT1(j, kt):
                """FC1 stationary operand for slot j (pair-ordered), k-tile kt."""
                if kt < KTA:
                    return w1ah[:, kt, j, :] if j < NHEAD else w1ar[:, j - NHEAD, kt, :]
                kb = kt - KTA
                return w1bh[:, kb, j, :] if j < NHEAD else w1br[:, j - NHEAD, kb, :]

            for ci in range(NCH):
                # ---- FC1 (proj/gate pair per iteration) + SwiGLU ----
                act = apool.tile([P, KT2, CH], BF16, tag="act", name=f"act{ci}")
                for mt in range(MT1H):
                    ps_p = pspool.tile([P, 512], F32, tag="ps", name=f"psp{ci}_{mt}")
                    ps_g = pspool.tile([P, 512], F32, tag="ps", name=f"psg{ci}_{mt}")
                    if ci == 0 and mt < 2:
                        # Fine phase: consume per-kt in arrival order, with
                        # warmup matmuls filling the DMA-pacing gaps.
                        for kt in range(KT1):
                            for ps, pg in ((ps_p, 0), (ps_g, 1)):
                                nc.tensor.matmul(
                                    ps[:, :CH],
                                    lhsT1(2 * mt + pg, kt),
                                    xt[:, kt, ci, :],
                                    start=(kt == 0),
                                    stop=(kt == KT1 - 1),
                                )
                            if mt == 0 and kt < NWARM_MID:
                                warm(1)
                    else:
                        for ps, pg in ((ps_p, 0), (ps_g, 1)):
                            for kt in range(KT1):
                                nc.tensor.matmul(
                                    ps[:, :CH],
                                    lhsT1(2 * mt + pg, kt),
                                    xt[:, kt, ci, :],
                                    start=(kt == 0),
                                    stop=(kt == KT1 - 1),
                                )
                    tmp = tpool.tile([P, CH], F32, tag="tmp", name=f"tmp{ci}_{mt}")
                    # PSUM carries 128*fc1 — the silu input-scale descales.
                    nc.scalar.activation(
                        tmp[:], ps_p[:, :CH], mybir.ActivationFunctionType.Silu,
                        scale=inv_scale,
                    )
                    nc.vector.tensor_mul(act[:, mt], tmp[:], ps_g[:, :CH])

                # ---- FC2 ----
                # act carries 128*act_true; the final copy descales by 1/128.
                # Output copies live on DVE; output DMA triggers on scalar.
                for m2 in range(MT2):
                    ps_o = pspool.tile([P, 512], F32, tag="ps", name=f"pso{ci}_{m2}")
                    o_sb = opool.tile([P, CH], BF16, tag="o", name=f"osb{ci}_{m2}")
                    if ci == NCH - 1 and m2 == MT2 - 1:
                        # Final m-tile: two independent half-column chains so
                        # the first half drains (copy+DMA) while the PE runs
                        # the second half — halves the post-last-matmul tail.
                        # Separate PSUM tiles: tile-granular WAR tracking
                        # would otherwise stall chain B behind chain A's copy.
                        ps_b = pspool.tile([P, 512], F32, tag="ps", name="psoB")
                        h = CH // 2
                        for ps, (c0, c1) in ((ps_o, (0, h)), (ps_b, (h, CH))):
                            for kt2 in range(KT2):
                                nc.tensor.matmul(
                                    ps[:, c0:c1],
                                    w2[:, m2, kt2, :],
                                    act[:, kt2, c0:c1],
                                    start=(kt2 == 0),
                                    stop=(kt2 == KT2 - 1),
                                )
                            nc.vector.tensor_scalar_mul(
                                o_sb[:, c0:c1], ps[:, c0:c1], inv_scale
                            )
                            nc.scalar.dma_start(
                                out=out_d[m2, ci, :, c0:c1], in_=o_sb[:, c0:c1]
                            )
                    else:
                        for kt2 in range(KT2):
                            nc.tensor.matmul(
                                ps_o[:, :CH],
                                w2[:, m2, kt2, :],
                                act[:, kt2, :],
                                start=(kt2 == 0),
                                stop=(kt2 == KT2 - 1),
                            )
                        nc.vector.tensor_scalar_mul(o_sb[:], ps_o[:, :CH], inv_scale)
                        nc.scalar.dma_start(out=out_d[m2, ci], in_=o_sb[:])

    nc.compile()
    return nc


def _get_graph(NCH: int, CH: int) -> bass.Bass:
    key = (NCH, CH)
    if key not in _graph_cache:
        _graph_cache[key] = _build(NCH, CH)
    return _graph_cache[key]


def _route(router_logits: np.ndarray):
    """Top-2 + softmax, exactly matching jax.lax.top_k tie-breaking."""
    idx = np.argsort(-router_logits, axis=-1, kind="stable")[:, :TOPK]
    tl = np.take_along_axis(router_logits, idx, axis=-1)
    ex = np.exp(tl - tl.max(-1, keepdims=True))
    sc = (ex / ex.sum(-1, keepdims=True)).astype(np.float32)
    return idx, sc


def run(hidden_states, router_logits, w1, w2, trace=False, trace_kwargs=None):
    hs = np.asarray(hidden_states, dtype=np.float32)
    rl = np.asarray(router_logits, dtype=np.float32)
    w1 = np.asarray(w1, dtype=np.float32)
    w2 = np.asarray(w2, dtype=np.float32)
    N, D = hs.shape

    idx, sc = _route(rl)

    tok_lists = []
    for e in range(NUM_EXPERTS):
        toks, slots = np.nonzero(idx == e)
        tok_lists.append((toks, slots))
    cmax = max(len(t) for t, _ in tok_lists)

    # Full-width (N=512) matmuls stream ~5% fewer PE cycles than two ragged
    # chunks. When the capacity overhang past a 512 multiple is small, cap
    # the device capacity at the multiple and run the few overflow tokens
    # through a f32 numpy epilogue on the host (<= 64 rows per expert;
    # routing/combine already live there).
    if cmax > 512 and cmax % 512 <= 64:
        C_dev = 512 * (cmax // 512)
    else:
        C_dev = cmax
    NCH = max(1, -(-C_dev // 512))
    CH = -(-C_dev // (NCH * 2)) * 2  # chunk width, multiple of 2
    C_pad = CH * NCH

    in_maps = []
    for e in range(NUM_EXPERTS):
        toks = tok_lists[e][0][:C_pad]
        x = np.zeros((C_pad, D), np.float32)
        x[: len(toks)] = hs[toks]
        xt = x.T.reshape(KT1, P, NCH, CH).transpose(1, 0, 2, 3).astype(np_bf16)
        # w1 pre-scaled by 128; slot axis pair-interleaved (_W1_ORDER); kt
        # 0-3 e3m4, kt 4-7 bf16; slots 0-3 additionally kt-major ("head").
        w1s = w1[e] * W1SCALE
        a = w1s[: KTA * P].reshape(KTA, P, MT1, P).transpose(1, 0, 2, 3)
        a = a[:, :, _W1_ORDER].astype(np_e3m4)
        b = w1s[KTA * P :].reshape(KTA, P, MT1, P).transpose(1, 0, 2, 3)
        b = b[:, :, _W1_ORDER].astype(np_bf16)
        w1ah = np.ascontiguousarray(a[:, :, :NHEAD])
        w1ar = np.ascontiguousarray(a[:, :, NHEAD:].transpose(0, 2, 1, 3))
        w1bh = np.ascontiguousarray(b[:, :, :NHEAD])
        w1br = np.ascontiguousarray(b[:, :, NHEAD:].transpose(0, 2, 1, 3))
        w2e = w2[e].reshape(KT2, P, MT2, P).transpose(1, 2, 0, 3).astype(np_bf16)
        in_maps.append(
            {
                "xt": xt,
                "w1ah": w1ah,
                "w1ar": w1ar,
                "w1bh": w1bh,
                "w1br": w1br,
                "w2": w2e,
            }
        )

    nc = _get_graph(NCH, CH)

    res = None
    for attempt in range(4):
        try:
            res = run_bass_kernel_spmd(
                nc,
                in_maps,
                core_ids=list(range(NCORES)),
                trace=trace,
                **(trace_kwargs or {}),
            )
            break
        except Exception:
            if attempt == 3:
                raise
            time.sleep(15 * (attempt + 1))

    out = np.zeros((N, D), np.float32)
    for e in range(NUM_EXPERTS):
        toks, slots = tok_lists[e]
        n_dev = min(len(toks), C_pad)
        oT = np.asarray(res.results[e]["out"]).astype(np.float32)
        oT = oT.transpose(0, 2, 1, 3).reshape(HIDDEN, C_pad)
        out[toks[:n_dev]] += sc[toks[:n_dev], slots[:n_dev]][:, None] * oT[:, :n_dev].T
        if n_dev < len(toks):
            # f32 host epilogue for the few overflow tokens past capacity
            ot, osl = toks[n_dev:], slots[n_dev:]
            h = hs[ot] @ w1[e]
            proj, gate = h[:, :INTER], h[:, INTER:]
            o = (proj / (1.0 + np.exp(-proj)) * gate) @ w2[e]
            out[ot] += sc[ot, osl][:, None] * o
    return out, res


def kernel(hidden_states, router_logits, w1, w2):
    out, _ = run(hidden_states, router_logits, w1, w2)
    return out


# revision 3
# speedup vs baseline: 1.0475x; 1.0475x over previous
"""MoE (AriaExperts) Trainium2 kernel — expert parallelism across 8 NeuronCores.

Strategy:
  - Host: top-2 routing + softmax over [2048, 8] logits (tiny), build the
    per-expert token batches (the "all-to-all" is realized at input
    distribution time), and the weighted scatter-add combine at the end.
  - Device (SPMD, 1 expert per core): dense GEMM chain in transposed
    activation layout so both matmuls consume the expert weights directly
    as the stationary (lhsT) operand with zero on-device transposes:
        H^T  = W1^T-tiles @ X^T      [2*INTER, C]
        actT = silu(projT) * gateT   [INTER, C]
        outT = W2-tiles   @ actT     [HIDDEN, C]
    All matmul dtypes are 1-cycle/row; f32 PSUM accumulation.

  Trace-driven design (baseline v1 at 102.4 us: first real MM 14.1 us
  gated by delivery of pair-0's 1.78 MB over the slowly-ramping DMA
  rings; MM stream then runs at the 216 ns/MM warm peak with <1.5 us of
  stalls; ~4.8 us tail):
    - w1 k-tiles 0-3 are shipped and consumed as e3m4 fp8 (4-bit
      mantissa, 1 cycle/row like bf16, FWL still applies); k-tiles 4-7
      stay bf16. Both halves are pre-scaled by 128 on the host (lossless
      for bf16, centers e3m4's tiny exponent range) so one PSUM chain
      mixes them; the 1/128 descale rides the silu input-scale and the
      FC2 output copy multiplier — zero extra ops. w1 drops 8 -> 6 MB,
      pair-0's critical head 1.78 -> 1.38 MB. Simulated end-to-end
      rel-err 1.40e-2 vs the 2e-2 gate (bf16 baseline 4.4e-3); HW
      matched the simulation to 3 digits.
    - Chunky DMA only: fine-grained (<=128 KB) slicing was measured (v2)
      to collapse early ring throughput (1.0 MB by 13 us vs 1.5 MB) and
      duty-cycle the PE so the HAM clock-gate stayed at 1.2 GHz until
      26 us. Big per-slot-group transfers in consumption order, split
      across the sync+scalar HWDGE rings. GpSimd's SWDGE measured only
      ~20-40 GB/s — it carries nothing pair-0-critical, just the last
      slot group and a w2 half.
    - PSUM pool = 8 x 1-bank [128,512] slots so 4 FC1 proj/gate pairs
      can be in flight; SwiGLU readout lags PE by several pairs without
      blocking PSUM recycling.
    - ACT (scalar) engine runs ONLY the 16 silus (plus a dummy 8-element
      silu right after boot to force the SILU ACT_TABLE_LOAD during the
      DMA window); FC2 PSUM->SBUF copies live on DVE; output DMA
      triggers on scalar whose completion-semaphore lanes are fresh.
    - PE warmup: NWARM matmuls on a memset tile flip the HAM clock-gate
      (1.2 -> 2.4 GHz needs ~3.4 us of sustained busy) and bridge until
      pair-0 data lands.
    - Final FC2 m-tile splits into two independent half-column chains so
      the first half drains (copy+DMA) while the PE runs the second.
"""

import time

import numpy as np
import ml_dtypes

import concourse.bass as bass
import concourse.bacc as bacc
import concourse.mybir as mybir
import concourse.tile as tile
from concourse.bass_utils import run_bass_kernel_spmd

NUM_TOKENS = 2048
HIDDEN = 1024
INTER = 2048
NUM_EXPERTS = 8
TOPK = 2
NCORES = 8
P = 128
KT1 = HIDDEN // P         # 8  k-tiles (FC1 contraction)
KTA = KT1 // 2            # 4  e3m4 k-tiles (kt 0-3)
MT1 = 2 * INTER // P      # 32 m-tiles (FC1 output rows = proj+gate)
MT1H = INTER // P         # 16 proj/gate pair count
KT2 = INTER // P          # 16 k-tiles (FC2 contraction)
MT2 = HIDDEN // P         # 8  m-tiles (FC2 output rows)
W1SCALE = 128.0           # host-side w1 pre-scale (both halves)

BF16 = mybir.dt.bfloat16
F8E3 = mybir.dt.float8e3
F32 = mybir.dt.float32
np_bf16 = ml_dtypes.bfloat16
np_e3m4 = ml_dtypes.float8_e3m4

# [0, 16, 1, 17, ...] — interleave proj/gate m-tiles into adjacent pairs
_W1_ORDER = np.arange(MT1).reshape(2, MT1H).T.reshape(-1)

NWARM = 11  # warmup matmuls: HAM flip (~3.4 us busy) + bridge until pair-0
            # data (~1.38 MB) lands ~12.6-13 us. 11 x 427 ns from ~8.5 us
            # ends ~13.2 with the tail self-compressing at the warm clock.

_graph_cache: dict = {}


def _build(NCH: int, CH: int) -> bass.Bass:
    """Per-core Bass graph for capacity C_pad = NCH * CH (CH <= 512)."""
    nc = bacc.Bacc("TRN2", target_bir_lowering=False, debug=False)

    xt_d = nc.declare_dram_parameter("xt", [P, KT1, NCH, CH], BF16, isOutput=False)
    # w1 split by contraction half: kt 0-3 e3m4, kt 4-7 bf16; slot axis
    # pair-interleaved (_W1_ORDER) so pair p's two m-tiles are adjacent.
    w1a_d = nc.declare_dram_parameter("w1a", [P, MT1, KTA, P], F8E3, isOutput=False)
    w1b_d = nc.declare_dram_parameter("w1b", [P, MT1, KTA, P], BF16, isOutput=False)
    w2_d = nc.declare_dram_parameter("w2", [P, MT2, KT2, P], BF16, isOutput=False)
    # bf16 output: halves the output DMA on the kernel tail; the host-side
    # combine upcasts to f32 (adds ~0.2% rounding — well within the gate).
    out_d = nc.declare_dram_parameter("out", [MT2, NCH, P, CH], BF16, isOutput=True)

    inv_scale = 1.0 / W1SCALE

    with tile.TileContext(nc) as tc:
        with (
            tc.tile_pool(name="weights", bufs=1) as wpool,
            tc.tile_pool(name="xin", bufs=1) as xpool,
            tc.tile_pool(name="actp", bufs=2) as apool,
            tc.tile_pool(name="tmp", bufs=4) as tpool,
            tc.tile_pool(name="osb", bufs=4) as opool,
            tc.tile_pool(name="psum", bufs=8, space="PSUM") as pspool,
        ):
            xt = xpool.tile([P, KT1, NCH, CH], BF16, tag="xt")
            w1a = wpool.tile([P, MT1, KTA, P], F8E3, tag="w1a")
            w1b = wpool.tile([P, MT1, KTA, P], BF16, tag="w1b")
            w2 = wpool.tile([P, MT2, KT2, P], BF16, tag="w2")
            dummy = xpool.tile([P, 640], BF16, tag="dummy")

            # Memset on DVE (idle at boot) so the warmup matmuls start as
            # early as possible.
            nc.vector.memset(dummy[:], 0.0)

            # PE warmup on the memset tile while inputs stream in.
            warm_ps = pspool.tile([P, 512], F32, tag="ps", name="warmps")
            for _ in range(NWARM):
                nc.tensor.matmul(
                    warm_ps[:, :], dummy[:, :128], dummy[:, 128:640],
                    start=True, stop=True,
                )

            # Force the SILU activation-table load (~1.3 us, non-blocking)
            # right at engine boot, overlapped with the first input DMAs.
            tdum = tpool.tile([P, 8], F32, tag="tmp", name="tdum")
            nc.scalar.activation(
                tdum[:], dummy[:, :8], mybir.ActivationFunctionType.Silu
            )

            # ---- input DMA triggers ----
            # Chunky transfers in consumption order across both HWDGE rings
            # (sync gets even pairs + xt k-tiles 0-3, scalar odd pairs + xt
            # k-tiles 4-7); per-slot-group e3m4 chunk first, bf16 second so
            # each pair's kt 0..7 chain finds its halves in arrival order.
            # GpSimd (slow SWDGE) carries only the last slot group and a w2
            # half — nothing pair-0/1-critical.
            nc.sync.dma_start(out=w1a[:, 0:1], in_=w1a_d[:, 0:1])
            nc.scalar.dma_start(out=w1a[:, 1:2], in_=w1a_d[:, 1:2])
            nc.sync.dma_start(out=w1b[:, 0:1], in_=w1b_d[:, 0:1])
            nc.scalar.dma_start(out=w1b[:, 1:2], in_=w1b_d[:, 1:2])
            nc.sync.dma_start(out=xt[:, 0:4, 0], in_=xt_d[:, 0:4, 0])
            nc.scalar.dma_start(out=xt[:, 4:8, 0], in_=xt_d[:, 4:8, 0])
            nc.sync.dma_start(out=w1a[:, 2:3], in_=w1a_d[:, 2:3])
            nc.scalar.dma_start(out=w1a[:, 3:4], in_=w1a_d[:, 3:4])
            nc.sync.dma_start(out=w1b[:, 2:3], in_=w1b_d[:, 2:3])
            nc.scalar.dma_start(out=w1b[:, 3:4], in_=w1b_d[:, 3:4])
            nc.sync.dma_start(out=w1a[:, 4:6], in_=w1a_d[:, 4:6])
            nc.scalar.dma_start(out=w1a[:, 6:8], in_=w1a_d[:, 6:8])
            nc.sync.dma_start(out=w1b[:, 4:6], in_=w1b_d[:, 4:6])
            nc.scalar.dma_start(out=w1b[:, 6:8], in_=w1b_d[:, 6:8])
            nc.sync.dma_start(out=w1a[:, 8:10], in_=w1a_d[:, 8:10])
            nc.scalar.dma_start(out=w1a[:, 10:12], in_=w1a_d[:, 10:12])
            nc.sync.dma_start(out=w1b[:, 8:10], in_=w1b_d[:, 8:10])
            nc.scalar.dma_start(out=w1b[:, 10:12], in_=w1b_d[:, 10:12])
            nc.sync.dma_start(out=w1a[:, 12:16], in_=w1a_d[:, 12:16])
            nc.scalar.dma_start(out=w1a[:, 16:20], in_=w1a_d[:, 16:20])
            nc.sync.dma_start(out=w1b[:, 12:16], in_=w1b_d[:, 12:16])
            nc.scalar.dma_start(out=w1b[:, 16:20], in_=w1b_d[:, 16:20])
            nc.sync.dma_start(out=w1a[:, 20:26], in_=w1a_d[:, 20:26])
            nc.scalar.dma_start(out=w2[:, MT2 // 2 :], in_=w2_d[:, MT2 // 2 :])
            nc.sync.dma_start(out=w1b[:, 20:26], in_=w1b_d[:, 20:26])
            nc.gpsimd.dma_start(out=w1a[:, 26:32], in_=w1a_d[:, 26:32])
            nc.sync.dma_start(out=w1b[:, 26:32], in_=w1b_d[:, 26:32])
            nc.sync.dma_start(out=w2[:, : MT2 // 2], in_=w2_d[:, : MT2 // 2])
            for ci in range(1, NCH):
                nc.sync.dma_start(out=xt[:, :, ci], in_=xt_d[:, :, ci])

            def lhsT1(j, kt):
                """FC1 stationary operand for slot j (pair-ordered), k-tile kt."""
                if kt < KTA:
                    return w1a[:, j, kt, :]
                return w1b[:, j, kt - KTA, :]

            for ci in range(NCH):
                # ---- FC1 (proj/gate pair per iteration) + SwiGLU ----
                act = apool.tile([P, KT2, CH], BF16, tag="act", name=f"act{ci}")
                for mt in range(MT1H):
                    ps_p = pspool.tile([P, 512], F32, tag="ps", name=f"psp{ci}_{mt}")
                    ps_g = pspool.tile([P, 512], F32, tag="ps", name=f"psg{ci}_{mt}")
                    for ps, pg in ((ps_p, 0), (ps_g, 1)):
                        for kt in range(KT1):
                            nc.tensor.matmul(
                                ps[:, :CH],
                                lhsT1(2 * mt + pg, kt),
                                xt[:, kt, ci, :],
                                start=(kt == 0),
                                stop=(kt == KT1 - 1),
                            )
                    tmp = tpool.tile([P, CH], F32, tag="tmp", name=f"tmp{ci}_{mt}")
                    # PSUM carries 128*fc1 — the silu input-scale descales.
                    nc.scalar.activation(
                        tmp[:], ps_p[:, :CH], mybir.ActivationFunctionType.Silu,
                        scale=inv_scale,
                    )
                    nc.vector.tensor_mul(act[:, mt], tmp[:], ps_g[:, :CH])

                # ---- FC2 ----
                # act carries 128*act_true; the final copy descales by 1/128.
                # Output copies live on DVE; output DMA triggers on scalar.
                for m2 in range(MT2):
                    ps_o = pspool.tile([P, 512], F32, tag="ps", name=f"pso{ci}_{m2}")
                    o_sb = opool.tile([P, CH], BF16, tag="o", name=f"osb{ci}_{m2}")
                    if ci == NCH - 1 and m2 == MT2 - 1:
                        # Final m-tile: two independent half-column chains so
                        # the first half drains (copy+DMA) while the PE runs
                        # the second half — halves the post-last-matmul tail.
                        # Separate PSUM tiles: tile-granular WAR tracking
                        # would otherwise stall chain B behind chain A's copy.
                        ps_b = pspool.tile([P, 512], F32, tag="ps", name="psoB")
                        h = CH // 2
                        for ps, (c0, c1) in ((ps_o, (0, h)), (ps_b, (h, CH))):
                            for kt2 in range(KT2):
                                nc.tensor.matmul(
                                    ps[:, c0:c1],
                                    w2[:, m2, kt2, :],
                                    act[:, kt2, c0:c1],
                                    start=(kt2 == 0),
                                    stop=(kt2 == KT2 - 1),
                                )
                            nc.vector.tensor_scalar_mul(
                                o_sb[:, c0:c1], ps[:, c0:c1], inv_scale
                            )
                            nc.scalar.dma_start(
                                out=out_d[m2, ci, :, c0:c1], in_=o_sb[:, c0:c1]
                            )
                    else:
                        for kt2 in range(KT2):
                            nc.tensor.matmul(
                                ps_o[:, :CH],
                                w2[:, m2, kt2, :],
                                act[:, kt2, :],
                                start=(kt2 == 0),
                                stop=(kt2 == KT2 - 1),
                            )
                        nc.vector.tensor_scalar_mul(o_sb[:], ps_o[:, :CH], inv_scale)
                        nc.scalar.dma_start(out=out_d[m2, ci], in_=o_sb[:])

    nc.compile()
    return nc


def _get_graph(NCH: int, CH: int) -> bass.Bass:
    key = (NCH, CH)
    if key not in _graph_cache:
        _graph_cache[key] = _build(NCH, CH)
    return _graph_cache[key]


def _route(router_logits: np.ndarray):
    """Top-2 + softmax, exactly matching jax.lax.top_k tie-breaking."""
    idx = np.argsort(-router_logits, axis=-1, kind="stable")[:, :TOPK]
    tl = np.take_along_axis(router_logits, idx, axis=-1)
    ex = np.exp(tl - tl.max(-1, keepdims=True))
    sc = (ex / ex.sum(-1, keepdims=True)).astype(np.float32)
    return idx, sc


def run(hidden_states, router_logits, w1, w2, trace=False, trace_kwargs=None):
    hs = np.asarray(hidden_states, dtype=np.float32)
    rl = np.asarray(router_logits, dtype=np.float32)
    w1 = np.asarray(w1, dtype=np.float32)
    w2 = np.asarray(w2, dtype=np.float32)
    N, D = hs.shape

    idx, sc = _route(rl)

    tok_lists = []
    for e in range(NUM_EXPERTS):
        toks, slots = np.nonzero(idx == e)
        tok_lists.append((toks, slots))
    cmax = max(len(t) for t, _ in tok_lists)

    # Full-width (N=512) matmuls stream ~5% fewer PE cycles than two ragged
    # chunks. When the capacity overhang past a 512 multiple is small, cap
    # the device capacity at the multiple and run the few overflow tokens
    # through a f32 numpy epilogue on the host (<= 64 rows per expert;
    # routing/combine already live there).
    if cmax > 512 and cmax % 512 <= 64:
        C_dev = 512 * (cmax // 512)
    else:
        C_dev = cmax
    NCH = max(1, -(-C_dev // 512))
    CH = -(-C_dev // (NCH * 2)) * 2  # chunk width, multiple of 2
    C_pad = CH * NCH

    in_maps = []
    for e in range(NUM_EXPERTS):
        toks = tok_lists[e][0][:C_pad]
        x = np.zeros((C_pad, D), np.float32)
        x[: len(toks)] = hs[toks]
        xt = x.T.reshape(KT1, P, NCH, CH).transpose(1, 0, 2, 3).astype(np_bf16)
        # w1 pre-scaled by 128; slot axis pair-interleaved (_W1_ORDER);
        # kt 0-3 e3m4 (w1a), kt 4-7 bf16 (w1b), both slot-major.
        w1s = w1[e] * W1SCALE
        a = w1s[: KTA * P].reshape(KTA, P, MT1, P).transpose(1, 2, 0, 3)
        w1ae = np.ascontiguousarray(a[:, _W1_ORDER]).astype(np_e3m4)
        b = w1s[KTA * P :].reshape(KTA, P, MT1, P).transpose(1, 2, 0, 3)
        w1be = np.ascontiguousarray(b[:, _W1_ORDER]).astype(np_bf16)
        w2e = w2[e].reshape(KT2, P, MT2, P).transpose(1, 2, 0, 3).astype(np_bf16)
        in_maps.append({"xt": xt, "w1a": w1ae, "w1b": w1be, "w2": w2e})

    nc = _get_graph(NCH, CH)

    res = None
    for attempt in range(4):
        try:
            res = run_bass_kernel_spmd(
                nc,
                in_maps,
                core_ids=list(range(NCORES)),
                trace=trace,
                **(trace_kwargs or {}),
            )
            break
        except Exception:
            if attempt == 3:
                raise
            time.sleep(15 * (attempt + 1))

    out = np.zeros((N, D), np.float32)
    for e in range(NUM_EXPERTS):
        toks, slots = tok_lists[e]
        n_dev = min(len(toks), C_pad)
        oT = np.asarray(res.results[e]["out"]).astype(np.float32)
        oT = oT.transpose(0, 2, 1, 3).reshape(HIDDEN, C_pad)
        out[toks[:n_dev]] += sc[toks[:n_dev], slots[:n_dev]][:, None] * oT[:, :n_dev].T
        if n_dev < len(toks):
            # f32 host epilogue for the few overflow tokens past capacity
            ot, osl = toks[n_dev:], slots[n_dev:]
            h = hs[ot] @ w1[e]
            proj, gate = h[:, :INTER], h[:, INTER:]
            o = (proj / (1.0 + np.exp(-proj)) * gate) @ w2[e]
            out[ot] += sc[ot, osl][:, None] * o
    return out, res


def kernel(hidden_states, router_logits, w1, w2):
    out, _ = run(hidden_states, router_logits, w1, w2)
    return out
